# revision 3
# baseline (speedup 1.0000x reference)
"""CEMSA v2: ACT-bound pipelined design on 8 trn2 cores.

Sharding: core = (batch b, head-half hh), as baseline.  Differences:

- x is host-transposed, bf16-converted and zero-PADDED into [2,128,68,68]
  images: no PE transposes, no DVE pad build.
- dw+pw conv fused into 18 dense accumulating PE matmuls per 512-chunk
  (stationary W_tap[ci,co] = pw2[co,ci]*dwt[ci,tap]): zero DVE conv work.
- SR conv as 18 diagonal PE matmuls.
- LN gamma/beta folded into kv weights / host-side proj bias; k bias is
  softmax-invariant (dropped); v bias folded into host-side proj bias.
- Attention pipelined per (chunk c of 512 n, unit u = (mt, head-pair)):
  S (2 row-packed MMs, K=32) -> PSUM ring slot [128,1024] (3-deep) ->
  ACT exp -> e bf16 -> O (col-packed M=32 MMs) + sum-of-exp (M=32 with
  an all-ones stationary, so every partition of a head's band carries the
  head's sum) into one shared PSUM bank per 256-col half.  ACT runs 128
  exp instructions back-to-back; everything else hides under them.
- Normalization is then a plain bf16 elementwise multiply with the
  reciprocal tile (no broadcast matmul); proj with otn stationary;
  y output bf16.

Host unshard: out[b] = y[2b] + y[2b+1] + (proj_b + proj_w @ b_v_eff).
"""

import os

import ml_dtypes
import numpy as np

import concourse.bass as bass
import concourse.tile as tile
from concourse import mybir
from concourse.bass_utils import run_bass_kernel_spmd

B, H, W, C, HEADS, SR = 4, 64, 64, 256, 8, 2
D = C // HEADS            # 32
N = H * W                 # 4096
M = (H // SR) * (W // SR) # 1024
SCALE = float(D) ** -0.5
EPS = 1e-6
NCORES = 8
PW = 68                   # padded image width/height (64 + 1 left + 3 right)
NCH = 8                   # n-chunks of 512
BF16NP = ml_dtypes.bfloat16

F32 = mybir.dt.float32
BF16 = mybir.dt.bfloat16

_CACHED = {}

# bf16 packed-constants column layout
_CPK_LAYOUT = [("wq", 18 * 128), ("dsr", 18 * 128), ("kvk", 256),
               ("kvv", 256), ("projT", 256), ("onesS", 32), ("onesc", 1)]
_CPK_OFF = {}
_o = 0
for _n, _w in _CPK_LAYOUT:
    _CPK_OFF[_n] = _o
    _o += _w
CPK_COLS = _o


class _SplitDrainTileContext(tile.TileContext):
    """This env's walrus rejects >1 sync wait on TPB_CTRL ops; TileContext's
    tail drain carries one wait per live semaphore.  Split the extras over a
    chain of SP NOPs (program order preserves semantics)."""

    MAX_WAITS = 1

    def _drain_and_barrier(self, tick_clock, wait_clock):
        nc = self.nc
        from concourse.tile import ScopedClock

        drain_inst = nc.sync.drain()
        wait_clock.add_sem_waits(
            drain_inst.ins, ScopedClock({None: tick_clock.global_clock})
        )
        si = drain_inst.ins.sync_info
        waits = list(si.on_wait) if si is not None and si.on_wait else []
        mw = self.MAX_WAITS
        if len(waits) > mw:
            si.on_wait = waits[:mw]
            rest = waits[mw:]
            for i in range(0, len(rest), mw):
                nop = nc.sync.nop()
                nsi = nop.ins.sync_info
                if nsi is None:
                    nop.ins.sync_info = type(si)(
                        on_wait=rest[i : i + mw], on_update=[]
                    )
                else:
                    nsi.on_wait = rest[i : i + mw]

        nc.all_engine_barrier()
        assert self.sems is not None
        popped = nc._tile_sem_poison_stack.pop()
        assert popped is self._sem_poison
        nc.clear_and_free_semaphores(list(self.sems.allocated().values()))
        nc.all_engine_barrier()


def _split_waits(nc):
    """This env's walrus allows only one sync-wait command per instruction
    (CTRL and LDWEIGHTS structs).  Move extra waits onto same-engine NOPs
    spliced immediately before the owning instruction."""
    k = 0
    for bb in nc.m.functions[0].blocks:
        new_insts = []
        for inst in bb.instructions:
            si = inst.sync_info
            waits = list(si.on_wait) if si is not None and si.on_wait else []
            if len(waits) > 1:
                for w in waits[:-1]:
                    nop = mybir.InstNoOp(name=f"wsplit-{k}", ins=[], outs=[])
                    k += 1
                    nop.engine = inst.engine
                    nop.sync_info = mybir.SyncInfo(on_wait=[w], on_update=[])
                    new_insts.append(nop)
                si.on_wait = [waits[-1]]
            new_insts.append(inst)
        bb.instructions[:] = new_insts
    return nc


def _build_nc(repeat=1, split_waits=True):
    nc = bass.Bass()

    params = {}
    for name, shape, dt in [
        ("xpad", [256, PW * PW], BF16),
        ("cpk", [128, CPK_COLS], BF16),
        ("ones1", [1, 128], BF16),
        ("qb", [128, 1], F32),
    ]:
        params[name] = nc.declare_dram_parameter(name, shape, dt, isOutput=False)
    params["y"] = nc.declare_dram_parameter("y", [N, C], BF16, isOutput=True)
    if os.environ.get("KERNEL_DEBUG"):
        for name, shape in [("dbg_q", [128, N]), ("dbg_kT", [128, M]),
                            ("dbg_v", [128, 8 * 128]), ("dbg_OT", [128, N]),
                            ("dbg_rcp", [128, 16 * 256]),
                            ("dbg_xsr", [256, M]), ("dbg_xln", [256, M])]:
            params[name] = nc.declare_dram_parameter(name, shape, BF16,
                                                     isOutput=True)

    with _SplitDrainTileContext(nc) as tc:
        with nc.allow_low_precision(reason="bf16 compute, tolerance 2e-2"):
            if repeat == 1:
                _emit(nc, tc, params)
            else:
                with tc.For_i(0, repeat):
                    _emit(nc, tc, params)
    if split_waits:
        _split_waits(nc)
    return nc


def _emit(nc, tc, t):
    Exp = mybir.ActivationFunctionType.Exp
    Sqrt = mybir.ActivationFunctionType.Sqrt
    mult = mybir.AluOpType.mult
    subtract = mybir.AluOpType.subtract

    with tc.tile_pool(name="consts", bufs=1) as cpool:
        cpk = cpool.tile([128, CPK_COLS], BF16, tag="cpk", name="cpk")
        ones1 = cpool.tile([1, 128], BF16, tag="ones1", name="ones1")
        qb = cpool.tile([128, 1], F32, tag="qb", name="qb")

        o = _CPK_OFF
        wq = [[cpk[:, o["wq"] + (tp * 2 + ct) * 128 :
                    o["wq"] + (tp * 2 + ct) * 128 + 128]
               for ct in range(2)] for tp in range(9)]
        dsr = [[cpk[:, o["dsr"] + (tp * 2 + ct) * 128 :
                     o["dsr"] + (tp * 2 + ct) * 128 + 128]
                for ct in range(2)] for tp in range(9)]
        kvk = [cpk[:, o["kvk"] + ct * 128 : o["kvk"] + ct * 128 + 128]
               for ct in range(2)]
        kvv = [cpk[:, o["kvv"] + ct * 128 : o["kvv"] + ct * 128 + 128]
               for ct in range(2)]
        projT = cpk[:, o["projT"] : o["projT"] + 256]
        onesS = cpk[:, o["onesS"] : o["onesS"] + 32]
        onesc = cpk[:, o["onesc"] : o["onesc"] + 1]

        nc.sync.dma_start(cpk[:], t["cpk"][:])
        nc.sync.dma_start(ones1[:], t["ones1"][:])
        nc.sync.dma_start(qb[:], t["qb"][:])

        with tc.tile_pool(name="live", bufs=1) as lp:
            pad = [lp.tile([128, PW, PW], BF16, tag=f"pad{ct}", name=f"pad{ct}")
                   for ct in range(2)]
            q_sb = lp.tile([128, N], BF16, tag="q", name="q")
            kT = lp.tile([128, M], BF16, tag="kT", name="kT")
            v_sb = lp.tile([128, 8, 128], BF16, tag="v", name="v")
            OT = lp.tile([128, N], BF16, tag="OT", name="OT")
            rcp = lp.tile([128, 16, 256], BF16, tag="rcp", name="rcp")
            xsr = [lp.tile([128, M], BF16, tag=f"xsr{ct}", name=f"xsr{ct}")
                   for ct in range(2)]
            xln = [lp.tile([128, M], BF16, tag=f"xln{ct}", name=f"xln{ct}")
                   for ct in range(2)]
            scr = [lp.tile([128, M], BF16, tag=f"scr{ct}", name=f"scr{ct}")
                   for ct in range(2)]
            mu16 = lp.tile([1, 2 * M], BF16, tag="mu16", name="mu16")
            muf = lp.tile([1, M], F32, tag="muf", name="muf")
            mu2 = lp.tile([1, M], F32, tag="mu2", name="mu2")
            var32 = lp.tile([1, M], F32, tag="var32", name="var32")

            xview = t["xpad"].rearrange("(ct p) f -> ct p f", p=128)
            for ct in range(2):
                nc.sync.dma_start(
                    pad[ct][:].rearrange("p a b -> p (a b)"), xview[ct]
                )

            with tc.tile_pool(name="qpsum", bufs=1, space="PSUM") as qp:

                def qconv(c):
                    ps = qp.tile([128, 512], F32, tag="qps", name="qps")
                    for tp in range(9):
                        dy, dx = tp // 3, tp % 3
                        for ct in range(2):
                            nc.tensor.matmul(
                                ps[:],
                                wq[tp][ct],
                                pad[ct][:, 8 * c + dy : 8 * c + dy + 8,
                                        dx : dx + 64],
                                start=(tp == 0 and ct == 0),
                                stop=(tp == 8 and ct == 1),
                            )
                    nc.vector.tensor_scalar_add(
                        q_sb[:, c * 512 : c * 512 + 512], ps[:], qb[:]
                    )

                # ---- phase 1: SR conv -> LN -> k/v
                with tc.tile_pool(name="srp", bufs=2, space="PSUM") as srp:
                    for ct in range(2):
                        v5 = pad[ct].rearrange(
                            "p (hh h2) (ww w2) -> p hh h2 ww w2", h2=2, w2=2
                        )
                        ps = srp.tile([128, M], F32, tag="srps", name="srps")
                        for half in range(2):
                            for tp in range(9):
                                dy, dx = tp // 3, tp % 3
                                h0, w0 = dy // 2, dx // 2
                                rhs = v5[:, h0 + half * 16 : h0 + half * 16 + 16,
                                         dy % 2, w0 : w0 + 32, dx % 2]
                                nc.tensor.matmul(
                                    ps[:, half * 512 : half * 512 + 512],
                                    dsr[tp][ct],
                                    rhs,
                                    start=(tp == 0),
                                    stop=(tp == 8),
                                )
                        nc.vector.tensor_copy(xsr[ct][:], ps[:])

                qconv(0)

                # LN stats: mean, mean-square via ones matmuls
                with tc.tile_pool(name="lnp", bufs=1, space="PSUM") as lnp:
                    mean_ps = lnp.tile([1, M], F32, tag="mean", name="mean")
                    msq_ps = lnp.tile([1, M], F32, tag="msq", name="msq")
                    for ct in range(2):
                        nc.vector.tensor_tensor(
                            scr[ct][:], xsr[ct][:], xsr[ct][:], op=mult
                        )
                    for half in range(2):
                        for ct in range(2):
                            nc.tensor.matmul(
                                mean_ps[:, half * 512 : half * 512 + 512],
                                onesc,
                                xsr[ct][:, half * 512 : half * 512 + 512],
                                start=(ct == 0),
                                stop=(ct == 1),
                            )
                            nc.tensor.matmul(
                                msq_ps[:, half * 512 : half * 512 + 512],
                                onesc,
                                scr[ct][:, half * 512 : half * 512 + 512],
                                start=(ct == 0),
                                stop=(ct == 1),
                            )
                    nc.vector.tensor_copy(muf[:], mean_ps[:])
                    nc.vector.tensor_copy(mu16[:, 0:M], muf[:])
                    nc.vector.tensor_tensor(mu2[:], muf[:], muf[:], op=mult)
                    nc.vector.tensor_tensor(var32[:], msq_ps[:], mu2[:], op=subtract)
                    nc.vector.tensor_scalar_add(var32[:], var32[:], EPS)
                    nc.scalar.activation(mu2[:], var32[:], Sqrt)
                    nc.vector.reciprocal(mu16[:, M : 2 * M], mu2[:])

                qconv(1)

                # broadcast mu/inv over partitions; apply LN (g/b folded out)
                with tc.tile_pool(name="bcp", bufs=1, space="PSUM") as bcp:
                    mu_b = bcp.tile([128, M], F32, tag="mu_b", name="mu_b")
                    inv_b = bcp.tile([128, M], F32, tag="inv_b", name="inv_b")
                    for half in range(2):
                        nc.tensor.matmul(
                            mu_b[:, half * 512 : half * 512 + 512],
                            ones1[:],
                            mu16[:, half * 512 : half * 512 + 512],
                            start=True, stop=True,
                        )
                        nc.tensor.matmul(
                            inv_b[:, half * 512 : half * 512 + 512],
                            ones1[:],
                            mu16[:, M + half * 512 : M + half * 512 + 512],
                            start=True, stop=True,
                        )
                    for ct in range(2):
                        nc.vector.tensor_tensor(
                            scr[ct][:], xsr[ct][:], mu_b[:], op=subtract
                        )
                        nc.vector.tensor_tensor(
                            xln[ct][:], scr[ct][:], inv_b[:], op=mult
                        )

                # k^T and v
                with tc.tile_pool(name="kvp", bufs=1, space="PSUM") as kvp, \
                     tc.tile_pool(name="vp", bufs=2, space="PSUM") as vp:
                    kps = kvp.tile([128, M], F32, tag="kps", name="kps")
                    for half in range(2):
                        for ct in range(2):
                            nc.tensor.matmul(
                                kps[:, half * 512 : half * 512 + 512],
                                kvk[ct],
                                xln[ct][:, half * 512 : half * 512 + 512],
                                start=(ct == 0),
                                stop=(ct == 1),
                            )
                    nc.vector.tensor_copy(kT[:], kps[:])
                    for mc in range(8):
                        # full-bank tile: PE-write + DVE-read of the same
                        # PSUM bank is fatal, so never share a bank
                        ps = vp.tile([128, 512], F32, tag="vps", name="vps")
                        for ct in range(2):
                            nc.tensor.matmul(
                                ps[:, 0:128],
                                xln[ct][:, mc * 128 : mc * 128 + 128],
                                kvv[ct],
                                start=(ct == 0),
                                stop=(ct == 1),
                            )
                        nc.vector.tensor_copy(v_sb[:, mc, :], ps[:, 0:128])

                # ---- attention ----
                with (
                    tc.tile_pool(name="sp", bufs=3, space="PSUM") as sp,
                    tc.tile_pool(name="osp", bufs=1, space="PSUM") as osp,
                    tc.tile_pool(name="ep", bufs=24) as ep,
                ):
                    es = {}  # (mt, p) -> e tile for the current chunk

                    def sexp_unit(c, u):
                        mt, p = u // 2, u % 2
                        s_t = sp.tile([128, 1024], F32, tag="s", name="s")
                        for hi in range(2):
                            h = p * 2 + hi
                            nc.tensor.matmul(
                                s_t[:, hi * 512 : hi * 512 + 512],
                                kT[32 * h : 32 * h + 32,
                                   mt * 128 : mt * 128 + 128],
                                q_sb[32 * h : 32 * h + 32,
                                     c * 512 : c * 512 + 512],
                                start=True, stop=True,
                                tile_position=(32 * h, 0),
                            )
                        e_t = ep.tile([128, 1024], BF16, tag="e", name="e")
                        nc.scalar.activation(e_t[:], s_t[:], Exp, scale=SCALE)
                        es[(c, mt, p)] = e_t

                    def oburst(c, f):
                        os_t = osp.tile([128, 512], F32, tag="os", name="os")
                        for u in range(16):
                            mt, p = u // 2, u % 2
                            e_t = es[(c, mt, p)]
                            for hi in range(2):
                                h = p * 2 + hi
                                rhs = e_t[:, hi * 512 + f * 256 :
                                          hi * 512 + f * 256 + 256]
                                # One start per (partition band x bank): it
                                # clears has_written for the whole bank row,
                                # so the sums matmul (start=False) overwrites
                                # on first touch and accumulates after.
                                nc.tensor.matmul(
                                    os_t[32 * h : 32 * h + 32, 0:256],
                                    v_sb[:, mt, 32 * h : 32 * h + 32],
                                    rhs,
                                    start=(mt == 0),
                                    stop=(mt == 7),
                                    tile_position=(0, 32 * h),
                                    skip_group_check=True,
                                )
                                nc.tensor.matmul(
                                    os_t[32 * h : 32 * h + 32, 256:512],
                                    onesS,
                                    rhs,
                                    start=False,
                                    stop=(mt == 7),
                                    tile_position=(0, 32 * h),
                                    skip_group_check=True,
                                )
                        # evict: O^T slice + reciprocal of the sums
                        nc.vector.tensor_copy(
                            OT[:, c * 512 + f * 256 : c * 512 + f * 256 + 256],
                            os_t[:, 0:256],
                        )
                        nc.vector.reciprocal(
                            rcp[:, c * 2 + f, :], os_t[:, 256:512]
                        )
                        if f == 1:
                            for u in range(16):
                                es.pop((c, u // 2, u % 2))

                    for c in range(NCH):
                        for u in range(4):
                            sexp_unit(c, u)
                        if c >= 1:
                            oburst(c - 1, 0)
                        for u in range(4, 8):
                            sexp_unit(c, u)
                        if c + 2 < NCH:
                            qconv(c + 2)
                        if c >= 1:
                            oburst(c - 1, 1)
                        for u in range(8, 16):
                            sexp_unit(c, u)
                    oburst(NCH - 1, 0)
                    oburst(NCH - 1, 1)

            # ---- normalize + project + write out ----
            rcp_flat = rcp[:].rearrange("p g f -> p (g f)")
            with (
                tc.tile_pool(name="yp", bufs=1, space="PSUM") as ypp,
                tc.tile_pool(name="otn", bufs=3) as otnp,
                tc.tile_pool(name="ytp", bufs=2) as ytp,
            ):
                for f0 in range(4):
                    otn = otnp.tile([128, 1024], BF16, tag="otn", name="otn")
                    nc.vector.tensor_tensor(
                        otn[:], OT[:, f0 * 1024 : f0 * 1024 + 1024],
                        rcp_flat[:, f0 * 1024 : f0 * 1024 + 1024],
                        op=mult,
                    )
                    y_ps = ypp.tile([128, 2048], F32, tag="yps", name="yps")
                    for sub in range(8):
                        nc.tensor.matmul(
                            y_ps[:, sub * 256 : sub * 256 + 256],
                            otn[:, sub * 128 : sub * 128 + 128],
                            projT,
                            start=True, stop=True,
                        )
                    yt = ytp.tile([128, 2048], BF16, tag="yt", name="yt")
                    nc.scalar.copy(yt[:], y_ps[:])
                    nc.sync.dma_start(
                        t["y"].rearrange("(f0 nt p) c -> f0 p nt c", f0=4, p=128)[f0],
                        yt[:].rearrange("p (nt c) -> p nt c", c=256),
                    )

            if os.environ.get("KERNEL_DEBUG"):
                nc.sync.dma_start(t["dbg_q"][:], q_sb[:])
                nc.sync.dma_start(t["dbg_kT"][:], kT[:])
                nc.sync.dma_start(t["dbg_v"][:],
                                  v_sb[:].rearrange("p a b -> p (a b)"))
                nc.sync.dma_start(t["dbg_OT"][:], OT[:])
                nc.sync.dma_start(t["dbg_rcp"][:],
                                  rcp[:].rearrange("p a b -> p (a b)"))
                for ct in range(2):
                    nc.sync.dma_start(
                        t["dbg_xsr"].rearrange("(ct p) f -> ct p f", p=128)[ct],
                        xsr[ct][:])
                    nc.sync.dma_start(
                        t["dbg_xln"].rearrange("(ct p) f -> ct p f", p=128)[ct],
                        xln[ct][:])


def _host_prep(x, dw_w, dw_b, pw_w, pw_b, sr_w, ln_g, ln_b, kv_w, kv_b,
               proj_w, proj_b):
    pw2 = pw_w[:, :, 0, 0]                       # [co, ci]
    dwt = dw_w[:, 0].reshape(C, 9)               # [ci, tap]
    srt = sr_w[:, 0].reshape(C, 9)
    qb_full = pw2 @ dw_b + pw_b                  # [C]
    kv_eff = kv_w * ln_g[None, :]                # fold LN gamma
    # v-bias + LN-beta contribution, folded into the host-side output bias
    b_v_eff = kv_b[C:] + kv_w[C:] @ ln_b         # [C] over j_v
    bias_out = proj_b + proj_w @ b_v_eff         # [C]

    o = _CPK_OFF
    consts = []
    for hh in range(2):
        co = slice(hh * 128, hh * 128 + 128)
        cpkv = np.zeros((128, CPK_COLS), np.float32)
        for ct in range(2):
            ci = slice(ct * 128, ct * 128 + 128)
            for tp in range(9):
                # W_tap^T[ci, co] = pw2[co, ci].T * dwt[ci, tap]
                cpkv[:, o["wq"] + (tp * 2 + ct) * 128 :
                        o["wq"] + (tp * 2 + ct) * 128 + 128] = (
                    pw2[co, ci].T * dwt[ci, tp][:, None]
                )
                cpkv[:, o["dsr"] + (tp * 2 + ct) * 128 :
                        o["dsr"] + (tp * 2 + ct) * 128 + 128] = np.diag(
                    srt[ci, tp]
                )
            # k rows for this hh: kv rows hh*128..hh*128+128 (k block)
            cpkv[:, o["kvk"] + ct * 128 : o["kvk"] + ct * 128 + 128] = (
                kv_eff[hh * 128 : hh * 128 + 128, ci].T
            )
            cpkv[:, o["kvv"] + ct * 128 : o["kvv"] + ct * 128 + 128] = (
                kv_eff[C + hh * 128 : C + hh * 128 + 128, ci].T
            )
        cpkv[:, o["projT"] : o["projT"] + 256] = proj_w[:, co].T
        cpkv[:, o["onesS"] : o["onesS"] + 32] = 1.0
        cpkv[:, o["onesc"]] = 1.0 / C
        consts.append(dict(
            cpk=np.ascontiguousarray(cpkv.astype(BF16NP)),
            qb=np.ascontiguousarray(qb_full[co])[:, None].astype(np.float32),
        ))

    shared = dict(ones1=np.ones((1, 128), BF16NP))

    # padded, transposed, bf16 images per batch
    xpads = []
    for b in range(B):
        img = x[b].T.reshape(2, 128, 64, 64)
        p = np.zeros((2, 128, PW, PW), np.float32)
        p[:, :, 1:65, 1:65] = img
        xpads.append(np.ascontiguousarray(
            p.reshape(256, PW * PW).astype(BF16NP)
        ))
    return consts, shared, xpads, bias_out


def kernel(x, dw_w, dw_b, pw_w, pw_b, sr_w, ln_g, ln_b, kv_w, kv_b,
           proj_w, proj_b):
    args = [np.asarray(a, np.float32) for a in
            (x, dw_w, dw_b, pw_w, pw_b, sr_w, ln_g, ln_b, kv_w, kv_b,
             proj_w, proj_b)]
    consts, shared, xpads, bias_out = _host_prep(*args)

    repeat = int(os.environ.get("KERNEL_REPEAT", "1"))
    key = f"nc{repeat}"
    if key not in _CACHED:
        _CACHED[key] = _build_nc(repeat)
    nc = _CACHED[key]

    in_maps = []
    for core in range(NCORES):
        b, hh = core // 2, core % 2
        in_maps.append(dict(xpad=xpads[b], **consts[hh], **shared))

    kw = {}
    if os.environ.get("KERNEL_TRACE"):
        kw = dict(trace=True)
    rr = run_bass_kernel_spmd(nc, in_maps, list(range(NCORES)), **kw)
    _CACHED["last"] = rr
    res = rr.results
    out = np.empty((B, N, C), np.float32)
    for b in range(B):
        out[b] = (res[2 * b]["y"].astype(np.float32)
                  + res[2 * b + 1]["y"].astype(np.float32)
                  + bias_out[None, :])
    return out


# revision 4
# speedup vs baseline: 1.0368x; 1.0368x over previous
"""CEMSA v2: ACT-bound pipelined design on 8 trn2 cores.

Sharding: core = (batch b, head-half hh), as baseline.  Differences:

- x is host-transposed, bf16-converted and zero-PADDED into [2,128,68,68]
  images: no PE transposes, no DVE pad build.
- dw+pw conv fused into 18 dense accumulating PE matmuls per 512-chunk
  (stationary W_tap[ci,co] = pw2[co,ci]*dwt[ci,tap]): zero DVE conv work.
- SR conv as 18 diagonal PE matmuls.
- LN gamma/beta folded into kv weights / host-side proj bias; k bias is
  softmax-invariant (dropped); v bias folded into host-side proj bias.
- Attention pipelined per (chunk c of 512 n, unit u = (mt, head-pair)):
  S (2 row-packed MMs, K=32) -> PSUM ring slot [128,1024] (3-deep) ->
  ACT exp -> e bf16 -> O (col-packed M=32 MMs) + sum-of-exp (M=32 with
  an all-ones stationary, so every partition of a head's band carries the
  head's sum) into one shared PSUM bank per 256-col half.  ACT runs 128
  exp instructions back-to-back; everything else hides under them.
- Normalization is then a plain bf16 elementwise multiply with the
  reciprocal tile (no broadcast matmul); proj with otn stationary;
  y output bf16.

Host unshard: out[b] = y[2b] + y[2b+1] + (proj_b + proj_w @ b_v_eff).
"""

import os

import ml_dtypes
import numpy as np

import concourse.bass as bass
import concourse.tile as tile
from concourse import mybir
from concourse.bass_utils import run_bass_kernel_spmd

B, H, W, C, HEADS, SR = 4, 64, 64, 256, 8, 2
D = C // HEADS            # 32
N = H * W                 # 4096
M = (H // SR) * (W // SR) # 1024
SCALE = float(D) ** -0.5
EPS = 1e-6
NCORES = 8
PW = 68                   # padded image width/height (64 + 1 left + 3 right)
NCH = 8                   # n-chunks of 512
BF16NP = ml_dtypes.bfloat16

F32 = mybir.dt.float32
BF16 = mybir.dt.bfloat16

_CACHED = {}

# bf16 packed-constants column layout
_CPK_LAYOUT = [("wq", 18 * 128), ("dsr", 18 * 128), ("kvk", 256),
               ("kvv", 256), ("projT", 256), ("onesS", 32), ("onesc", 1)]
_CPK_OFF = {}
_o = 0
for _n, _w in _CPK_LAYOUT:
    _CPK_OFF[_n] = _o
    _o += _w
CPK_COLS = _o


class _SplitDrainTileContext(tile.TileContext):
    """This env's walrus rejects >1 sync wait on TPB_CTRL ops; TileContext's
    tail drain carries one wait per live semaphore.  Split the extras over a
    chain of SP NOPs (program order preserves semantics)."""

    MAX_WAITS = 1

    def _drain_and_barrier(self, tick_clock, wait_clock):
        nc = self.nc
        from concourse.tile import ScopedClock

        drain_inst = nc.sync.drain()
        wait_clock.add_sem_waits(
            drain_inst.ins, ScopedClock({None: tick_clock.global_clock})
        )
        si = drain_inst.ins.sync_info
        waits = list(si.on_wait) if si is not None and si.on_wait else []
        mw = self.MAX_WAITS
        if len(waits) > mw:
            si.on_wait = waits[:mw]
            rest = waits[mw:]
            for i in range(0, len(rest), mw):
                nop = nc.sync.nop()
                nsi = nop.ins.sync_info
                if nsi is None:
                    nop.ins.sync_info = type(si)(
                        on_wait=rest[i : i + mw], on_update=[]
                    )
                else:
                    nsi.on_wait = rest[i : i + mw]

        nc.all_engine_barrier()
        assert self.sems is not None
        popped = nc._tile_sem_poison_stack.pop()
        assert popped is self._sem_poison
        nc.clear_and_free_semaphores(list(self.sems.allocated().values()))
        nc.all_engine_barrier()


def _split_waits(nc):
    """This env's walrus allows only one sync-wait command per instruction
    (CTRL and LDWEIGHTS structs).  Move extra waits onto same-engine NOPs
    spliced immediately before the owning instruction."""
    k = 0
    for bb in nc.m.functions[0].blocks:
        new_insts = []
        for inst in bb.instructions:
            si = inst.sync_info
            waits = list(si.on_wait) if si is not None and si.on_wait else []
            if len(waits) > 1:
                for w in waits[:-1]:
                    nop = mybir.InstNoOp(name=f"wsplit-{k}", ins=[], outs=[])
                    k += 1
                    nop.engine = inst.engine
                    nop.sync_info = mybir.SyncInfo(on_wait=[w], on_update=[])
                    new_insts.append(nop)
                si.on_wait = [waits[-1]]
            new_insts.append(inst)
        bb.instructions[:] = new_insts
    return nc


def _build_nc(repeat=1, split_waits=True):
    nc = bass.Bass()

    params = {}
    for name, shape, dt in [
        ("xpad", [256, PW * PW], BF16),
        ("cpk", [128, CPK_COLS], BF16),
        ("ones1", [1, 128], BF16),
        ("qb", [128, 1], F32),
    ]:
        params[name] = nc.declare_dram_parameter(name, shape, dt, isOutput=False)
    params["y"] = nc.declare_dram_parameter("y", [N, C], BF16, isOutput=True)
    if os.environ.get("KERNEL_DEBUG"):
        for name, shape in [("dbg_q", [128, N]), ("dbg_kT", [128, M]),
                            ("dbg_v", [128, 8 * 128]), ("dbg_OT", [128, N]),
                            ("dbg_rcp", [128, 16 * 256]),
                            ("dbg_xsr", [256, M]), ("dbg_xln", [256, M])]:
            params[name] = nc.declare_dram_parameter(name, shape, BF16,
                                                     isOutput=True)

    with _SplitDrainTileContext(nc) as tc:
        with nc.allow_low_precision(reason="bf16 compute, tolerance 2e-2"):
            if repeat == 1:
                _emit(nc, tc, params)
            else:
                with tc.For_i(0, repeat):
                    _emit(nc, tc, params)
    if split_waits:
        _split_waits(nc)
    return nc


def _emit(nc, tc, t):
    Exp = mybir.ActivationFunctionType.Exp
    Log = mybir.ActivationFunctionType.Ln
    mult = mybir.AluOpType.mult
    subtract = mybir.AluOpType.subtract

    with tc.tile_pool(name="consts", bufs=1) as cpool:
        cpk = cpool.tile([128, CPK_COLS], BF16, tag="cpk", name="cpk")
        ones1 = cpool.tile([1, 128], BF16, tag="ones1", name="ones1")
        qb = cpool.tile([128, 1], F32, tag="qb", name="qb")

        o = _CPK_OFF
        wq = [[cpk[:, o["wq"] + (tp * 2 + ct) * 128 :
                    o["wq"] + (tp * 2 + ct) * 128 + 128]
               for ct in range(2)] for tp in range(9)]
        dsr = [[cpk[:, o["dsr"] + (tp * 2 + ct) * 128 :
                     o["dsr"] + (tp * 2 + ct) * 128 + 128]
                for ct in range(2)] for tp in range(9)]
        kvk = [cpk[:, o["kvk"] + ct * 128 : o["kvk"] + ct * 128 + 128]
               for ct in range(2)]
        kvv = [cpk[:, o["kvv"] + ct * 128 : o["kvv"] + ct * 128 + 128]
               for ct in range(2)]
        projT = cpk[:, o["projT"] : o["projT"] + 256]
        onesS = cpk[:, o["onesS"] : o["onesS"] + 32]
        onesc = cpk[:, o["onesc"] : o["onesc"] + 1]

        nc.sync.dma_start(cpk[:], t["cpk"][:])
        nc.sync.dma_start(ones1[:], t["ones1"][:])
        nc.sync.dma_start(qb[:], t["qb"][:])

        with tc.tile_pool(name="live", bufs=1) as lp:
            pad = [lp.tile([128, PW, PW], BF16, tag=f"pad{ct}", name=f"pad{ct}")
                   for ct in range(2)]
            q_sb = lp.tile([128, N], BF16, tag="q", name="q")
            kT = lp.tile([128, M], BF16, tag="kT", name="kT")
            v_sb = lp.tile([128, 8, 128], BF16, tag="v", name="v")
            OT = lp.tile([128, N], BF16, tag="OT", name="OT")
            rcp = lp.tile([128, 16, 256], BF16, tag="rcp", name="rcp")
            xsr = [lp.tile([128, M], BF16, tag=f"xsr{ct}", name=f"xsr{ct}")
                   for ct in range(2)]
            xln = [lp.tile([128, M], BF16, tag=f"xln{ct}", name=f"xln{ct}")
                   for ct in range(2)]
            scr = [lp.tile([128, M], BF16, tag=f"scr{ct}", name=f"scr{ct}")
                   for ct in range(2)]
            mu16 = lp.tile([1, 2 * M], BF16, tag="mu16", name="mu16")
            muf = lp.tile([1, M], F32, tag="muf", name="muf")
            mu2 = lp.tile([1, M], F32, tag="mu2", name="mu2")
            var32 = lp.tile([1, M], F32, tag="var32", name="var32")

            xview = t["xpad"].rearrange("(ct p) f -> ct p f", p=128)
            for ct in range(2):
                nc.sync.dma_start(
                    pad[ct][:].rearrange("p a b -> p (a b)"), xview[ct]
                )

            with tc.tile_pool(name="qpsum", bufs=1, space="PSUM") as qp:

                def qconv(c):
                    ps = qp.tile([128, 512], F32, tag="qps", name="qps")
                    for tp in range(9):
                        dy, dx = tp // 3, tp % 3
                        for ct in range(2):
                            nc.tensor.matmul(
                                ps[:],
                                wq[tp][ct],
                                pad[ct][:, 8 * c + dy : 8 * c + dy + 8,
                                        dx : dx + 64],
                                start=(tp == 0 and ct == 0),
                                stop=(tp == 8 and ct == 1),
                            )
                    nc.vector.tensor_scalar_add(
                        q_sb[:, c * 512 : c * 512 + 512], ps[:], qb[:]
                    )

                # ---- phase 1: SR conv -> LN -> k/v
                with tc.tile_pool(name="srp", bufs=2, space="PSUM") as srp:
                    for ct in range(2):
                        v5 = pad[ct].rearrange(
                            "p (hh h2) (ww w2) -> p hh h2 ww w2", h2=2, w2=2
                        )
                        ps = srp.tile([128, M], F32, tag="srps", name="srps")
                        for half in range(2):
                            for tp in range(9):
                                dy, dx = tp // 3, tp % 3
                                h0, w0 = dy // 2, dx // 2
                                rhs = v5[:, h0 + half * 16 : h0 + half * 16 + 16,
                                         dy % 2, w0 : w0 + 32, dx % 2]
                                nc.tensor.matmul(
                                    ps[:, half * 512 : half * 512 + 512],
                                    dsr[tp][ct],
                                    rhs,
                                    start=(tp == 0),
                                    stop=(tp == 8),
                                )
                        nc.vector.tensor_copy(xsr[ct][:], ps[:])

                qconv(0)

                # LN stats: mean, mean-square via ones matmuls
                with tc.tile_pool(name="lnp", bufs=1, space="PSUM") as lnp:
                    mean_ps = lnp.tile([1, M], F32, tag="mean", name="mean")
                    msq_ps = lnp.tile([1, M], F32, tag="msq", name="msq")
                    for ct in range(2):
                        nc.vector.tensor_tensor(
                            scr[ct][:], xsr[ct][:], xsr[ct][:], op=mult
                        )
                    for half in range(2):
                        for ct in range(2):
                            nc.tensor.matmul(
                                mean_ps[:, half * 512 : half * 512 + 512],
                                onesc,
                                xsr[ct][:, half * 512 : half * 512 + 512],
                                start=(ct == 0),
                                stop=(ct == 1),
                            )
                            nc.tensor.matmul(
                                msq_ps[:, half * 512 : half * 512 + 512],
                                onesc,
                                scr[ct][:, half * 512 : half * 512 + 512],
                                start=(ct == 0),
                                stop=(ct == 1),
                            )
                    nc.vector.tensor_copy(muf[:], mean_ps[:])
                    nc.vector.tensor_copy(mu16[:, 0:M], muf[:])
                    nc.vector.tensor_tensor(mu2[:], muf[:], muf[:], op=mult)
                    nc.vector.tensor_tensor(var32[:], msq_ps[:], mu2[:], op=subtract)
                    nc.vector.tensor_scalar_add(var32[:], var32[:], EPS)
                    # 1/sqrt(v) = exp(-0.5*ln(v)): stays in the
                    # natural_log_exp activation-table set (no switches)
                    nc.scalar.activation(mu2[:], var32[:], Log)
                    nc.scalar.activation(mu16[:, M : 2 * M], mu2[:], Exp,
                                         scale=-0.5)

                qconv(1)

                # broadcast mu/inv over partitions; apply LN (g/b folded out)
                with tc.tile_pool(name="bcp", bufs=1, space="PSUM") as bcp:
                    mu_b = bcp.tile([128, M], F32, tag="mu_b", name="mu_b")
                    inv_b = bcp.tile([128, M], F32, tag="inv_b", name="inv_b")
                    for half in range(2):
                        nc.tensor.matmul(
                            mu_b[:, half * 512 : half * 512 + 512],
                            ones1[:],
                            mu16[:, half * 512 : half * 512 + 512],
                            start=True, stop=True,
                        )
                        nc.tensor.matmul(
                            inv_b[:, half * 512 : half * 512 + 512],
                            ones1[:],
                            mu16[:, M + half * 512 : M + half * 512 + 512],
                            start=True, stop=True,
                        )
                    for ct in range(2):
                        nc.vector.tensor_tensor(
                            scr[ct][:], xsr[ct][:], mu_b[:], op=subtract
                        )
                        nc.vector.tensor_tensor(
                            xln[ct][:], scr[ct][:], inv_b[:], op=mult
                        )

                # k^T and v
                with tc.tile_pool(name="kvp", bufs=1, space="PSUM") as kvp, \
                     tc.tile_pool(name="vp", bufs=2, space="PSUM") as vp:
                    kps = kvp.tile([128, M], F32, tag="kps", name="kps")
                    for half in range(2):
                        for ct in range(2):
                            nc.tensor.matmul(
                                kps[:, half * 512 : half * 512 + 512],
                                kvk[ct],
                                xln[ct][:, half * 512 : half * 512 + 512],
                                start=(ct == 0),
                                stop=(ct == 1),
                            )
                    nc.vector.tensor_copy(kT[:], kps[:])
                    for mc in range(8):
                        # full-bank tile: PE-write + DVE-read of the same
                        # PSUM bank is fatal, so never share a bank
                        ps = vp.tile([128, 512], F32, tag="vps", name="vps")
                        for ct in range(2):
                            nc.tensor.matmul(
                                ps[:, 0:128],
                                xln[ct][:, mc * 128 : mc * 128 + 128],
                                kvv[ct],
                                start=(ct == 0),
                                stop=(ct == 1),
                            )
                        nc.vector.tensor_copy(v_sb[:, mc, :], ps[:, 0:128])

                # ---- attention ----
                with (
                    tc.tile_pool(name="sp", bufs=3, space="PSUM") as sp,
                    tc.tile_pool(name="osp", bufs=1, space="PSUM") as osp,
                    tc.tile_pool(name="ep", bufs=12) as ep,
                ):
                    es = {}  # (c, mt) -> e tile [128, 2048]

                    def sexp_pair(c, mt):
                        e_t = ep.tile([128, 2048], BF16, tag="e", name="e")
                        for p in range(2):
                            s_t = sp.tile([128, 1024], F32, tag="s", name="s")
                            for hi in range(2):
                                h = p * 2 + hi
                                nc.tensor.matmul(
                                    s_t[:, hi * 512 : hi * 512 + 512],
                                    kT[32 * h : 32 * h + 32,
                                       mt * 128 : mt * 128 + 128],
                                    q_sb[32 * h : 32 * h + 32,
                                         c * 512 : c * 512 + 512],
                                    start=True, stop=True,
                                    tile_position=(32 * h, 0),
                                )
                            nc.scalar.activation(
                                e_t[:, p * 1024 : p * 1024 + 1024], s_t[:],
                                Exp, scale=SCALE)
                        es[(c, mt)] = e_t

                    def oburst(c, f):
                        os_t = osp.tile([128, 512], F32, tag="os", name="os")
                        for u in range(16):
                            mt, p = u // 2, u % 2
                            e_t = es[(c, mt)]
                            for hi in range(2):
                                h = p * 2 + hi
                                rhs = e_t[:, p * 1024 + hi * 512 + f * 256 :
                                          p * 1024 + hi * 512 + f * 256 + 256]
                                # One start per (partition band x bank): it
                                # clears has_written for the whole bank row,
                                # so the sums matmul (start=False) overwrites
                                # on first touch and accumulates after.
                                nc.tensor.matmul(
                                    os_t[32 * h : 32 * h + 32, 0:256],
                                    v_sb[:, mt, 32 * h : 32 * h + 32],
                                    rhs,
                                    start=(mt == 0),
                                    stop=(mt == 7),
                                    tile_position=(0, 32 * h),
                                    skip_group_check=True,
                                )
                                nc.tensor.matmul(
                                    os_t[32 * h : 32 * h + 32, 256:512],
                                    onesS,
                                    rhs,
                                    start=False,
                                    stop=(mt == 7),
                                    tile_position=(0, 32 * h),
                                    skip_group_check=True,
                                )
                        # evict: O^T slice + reciprocal of the sums
                        nc.vector.tensor_copy(
                            OT[:, c * 512 + f * 256 : c * 512 + f * 256 + 256],
                            os_t[:, 0:256],
                        )
                        nc.vector.reciprocal(
                            rcp[:, c * 2 + f, :], os_t[:, 256:512]
                        )
                        if f == 1:
                            for mt in range(8):
                                es.pop((c, mt))

                    for c in range(NCH):
                        for mt in range(2):
                            sexp_pair(c, mt)
                        if c >= 1:
                            oburst(c - 1, 0)
                        for mt in range(2, 4):
                            sexp_pair(c, mt)
                        if c + 2 < NCH:
                            qconv(c + 2)
                        if c >= 1:
                            oburst(c - 1, 1)
                        for mt in range(4, 8):
                            sexp_pair(c, mt)
                    oburst(NCH - 1, 0)
                    oburst(NCH - 1, 1)

            # ---- normalize + project + write out ----
            rcp_flat = rcp[:].rearrange("p g f -> p (g f)")
            with (
                tc.tile_pool(name="yp", bufs=1, space="PSUM") as ypp,
                tc.tile_pool(name="otn", bufs=3) as otnp,
                tc.tile_pool(name="ytp", bufs=2) as ytp,
            ):
                for f0 in range(4):
                    otn = otnp.tile([128, 1024], BF16, tag="otn", name="otn")
                    nc.vector.tensor_tensor(
                        otn[:], OT[:, f0 * 1024 : f0 * 1024 + 1024],
                        rcp_flat[:, f0 * 1024 : f0 * 1024 + 1024],
                        op=mult,
                    )
                    y_ps = ypp.tile([128, 2048], F32, tag="yps", name="yps")
                    for sub in range(8):
                        nc.tensor.matmul(
                            y_ps[:, sub * 256 : sub * 256 + 256],
                            otn[:, sub * 128 : sub * 128 + 128],
                            projT,
                            start=True, stop=True,
                        )
                    yt = ytp.tile([128, 2048], BF16, tag="yt", name="yt")
                    nc.vector.tensor_copy(yt[:], y_ps[:])
                    nc.sync.dma_start(
                        t["y"].rearrange("(f0 nt p) c -> f0 p nt c", f0=4, p=128)[f0],
                        yt[:].rearrange("p (nt c) -> p nt c", c=256),
                    )

            if os.environ.get("KERNEL_DEBUG"):
                nc.sync.dma_start(t["dbg_q"][:], q_sb[:])
                nc.sync.dma_start(t["dbg_kT"][:], kT[:])
                nc.sync.dma_start(t["dbg_v"][:],
                                  v_sb[:].rearrange("p a b -> p (a b)"))
                nc.sync.dma_start(t["dbg_OT"][:], OT[:])
                nc.sync.dma_start(t["dbg_rcp"][:],
                                  rcp[:].rearrange("p a b -> p (a b)"))
                for ct in range(2):
                    nc.sync.dma_start(
                        t["dbg_xsr"].rearrange("(ct p) f -> ct p f", p=128)[ct],
                        xsr[ct][:])
                    nc.sync.dma_start(
                        t["dbg_xln"].rearrange("(ct p) f -> ct p f", p=128)[ct],
                        xln[ct][:])


def _host_prep(x, dw_w, dw_b, pw_w, pw_b, sr_w, ln_g, ln_b, kv_w, kv_b,
               proj_w, proj_b):
    pw2 = pw_w[:, :, 0, 0]                       # [co, ci]
    dwt = dw_w[:, 0].reshape(C, 9)               # [ci, tap]
    srt = sr_w[:, 0].reshape(C, 9)
    qb_full = pw2 @ dw_b + pw_b                  # [C]
    kv_eff = kv_w * ln_g[None, :]                # fold LN gamma
    # v-bias + LN-beta contribution, folded into the host-side output bias
    b_v_eff = kv_b[C:] + kv_w[C:] @ ln_b         # [C] over j_v
    bias_out = proj_b + proj_w @ b_v_eff         # [C]

    o = _CPK_OFF
    consts = []
    for hh in range(2):
        co = slice(hh * 128, hh * 128 + 128)
        cpkv = np.zeros((128, CPK_COLS), np.float32)
        for ct in range(2):
            ci = slice(ct * 128, ct * 128 + 128)
            for tp in range(9):
                # W_tap^T[ci, co] = pw2[co, ci].T * dwt[ci, tap]
                cpkv[:, o["wq"] + (tp * 2 + ct) * 128 :
                        o["wq"] + (tp * 2 + ct) * 128 + 128] = (
                    pw2[co, ci].T * dwt[ci, tp][:, None]
                )
                cpkv[:, o["dsr"] + (tp * 2 + ct) * 128 :
                        o["dsr"] + (tp * 2 + ct) * 128 + 128] = np.diag(
                    srt[ci, tp]
                )
            # k rows for this hh: kv rows hh*128..hh*128+128 (k block)
            cpkv[:, o["kvk"] + ct * 128 : o["kvk"] + ct * 128 + 128] = (
                kv_eff[hh * 128 : hh * 128 + 128, ci].T
            )
            cpkv[:, o["kvv"] + ct * 128 : o["kvv"] + ct * 128 + 128] = (
                kv_eff[C + hh * 128 : C + hh * 128 + 128, ci].T
            )
        cpkv[:, o["projT"] : o["projT"] + 256] = proj_w[:, co].T
        cpkv[:, o["onesS"] : o["onesS"] + 32] = 1.0
        cpkv[:, o["onesc"]] = 1.0 / C
        consts.append(dict(
            cpk=np.ascontiguousarray(cpkv.astype(BF16NP)),
            qb=np.ascontiguousarray(qb_full[co])[:, None].astype(np.float32),
        ))

    shared = dict(ones1=np.ones((1, 128), BF16NP))

    # padded, transposed, bf16 images per batch
    xpads = []
    for b in range(B):
        img = x[b].T.reshape(2, 128, 64, 64)
        p = np.zeros((2, 128, PW, PW), np.float32)
        p[:, :, 1:65, 1:65] = img
        xpads.append(np.ascontiguousarray(
            p.reshape(256, PW * PW).astype(BF16NP)
        ))
    return consts, shared, xpads, bias_out


def kernel(x, dw_w, dw_b, pw_w, pw_b, sr_w, ln_g, ln_b, kv_w, kv_b,
           proj_w, proj_b):
    args = [np.asarray(a, np.float32) for a in
            (x, dw_w, dw_b, pw_w, pw_b, sr_w, ln_g, ln_b, kv_w, kv_b,
             proj_w, proj_b)]
    consts, shared, xpads, bias_out = _host_prep(*args)

    repeat = int(os.environ.get("KERNEL_REPEAT", "1"))
    key = f"nc{repeat}"
    if key not in _CACHED:
        _CACHED[key] = _build_nc(repeat)
    nc = _CACHED[key]

    in_maps = []
    for core in range(NCORES):
        b, hh = core // 2, core % 2
        in_maps.append(dict(xpad=xpads[b], **consts[hh], **shared))

    kw = {}
    if os.environ.get("KERNEL_TRACE"):
        kw = dict(trace=True)
    rr = run_bass_kernel_spmd(nc, in_maps, list(range(NCORES)), **kw)
    _CACHED["last"] = rr
    res = rr.results
    out = np.empty((B, N, C), np.float32)
    for b in range(B):
        out[b] = (res[2 * b]["y"].astype(np.float32)
                  + res[2 * b + 1]["y"].astype(np.float32)
                  + bias_out[None, :])
    return out
